# revision 1
# baseline (speedup 1.0000x reference)
"""Trainium2 Bass kernel for a 4-layer pre-LN transformer + GEGLU FFN.

Sharding: rows (batch*seq) split across 8 cores; cores 0-3 own batch 0,
cores 4-7 own batch 1 (512 rows each).  Attention needs full-sequence K/V
per batch element, so each 4-core group AllGathers its K/V shards per layer.

On-chip dataflow (per core, per layer):
  LN (natural [rows,C] layout, DVE stats + quake-rsqrt)
  h^T via DMA x-bar transpose (bf16)
  Q^T/K^T (transposed via lhsT=W), V (natural via lhsT=h^T); bf16 matmuls
  K/V -> DRAM bounce -> AllGather(group of 4) -> K^T full / V full in SBUF
  per head: scores^T = K^T.T @ Q^T (PSUM fp32) -> exp on ACT (scale=1/8)
            -> bf16 exp_scores; AV with lhsT=[V|1] gives o^T and softmax
            sums in one accumulation; normalize via DVE reciprocal
            + PE ones-broadcast; out-proj back to natural layout + residual.
  Head loop is software-pipelined: scores/exp of head h overlap the AV
  matmul chain of head h-1 so ACT never waits behind the in-order PE queue.
Final LN + GEGLU FFN (explicit tanh formula) + residual -> output.
"""

import numpy as np
import ml_dtypes

B, S, C = 2, 2048, 512
L, H, CH = 4, 8, 64
OD = 4 * CH  # 256
EPS = 1e-5

N_CORES = 8
GROUP = 4          # cores per batch element
ROWS = (B * S) // N_CORES  # 512 rows per core
P = 128
RT = ROWS // P     # 4 row tiles
CCH = C // P       # 4 chunks of the hidden/attention dim
KT = S // P        # 16 k tiles (full sequence)
KTO = ROWS // P    # 4 own k tiles
VW = H * (CH + 1)  # 520: V row layout, 64 cols + 1 ones col per head
SQRT_K = 0x5F3759DF

BF16 = ml_dtypes.bfloat16

_CACHE = {}


def _swz(w, pt):
    # [pt*128, N] -> [128, pt*N] with chunk-major free dim
    n = w.shape[1]
    return np.ascontiguousarray(
        w.reshape(pt, P, n).transpose(1, 0, 2).reshape(P, pt * n)
    )


def _build(flags, n_layers=L, fake_ag=False, debug=False):
    use_gamma, use_beta, use_bo, use_bg, use_bf = flags
    import concourse.bass as bass
    import concourse.bacc as bacc
    import concourse.mybir as mybir
    import concourse.tile as tile

    dt = mybir.dt
    AF = mybir.ActivationFunctionType
    OP = mybir.AluOpType

    nc = bacc.Bacc("TRN2", target_bir_lowering=False, debug=False,
                   num_devices=N_CORES)
    groups = [list(range(g * GROUP, (g + 1) * GROUP))
              for g in range(N_CORES // GROUP)]

    # ---- DRAM I/O ----
    x_d = nc.dram_tensor("x", [ROWS, C], dt.float32, kind="ExternalInput")
    wq_d = nc.dram_tensor("wq", [L, P, CCH * C], dt.bfloat16, kind="ExternalInput")
    wk_d = nc.dram_tensor("wk", [L, P, CCH * C], dt.bfloat16, kind="ExternalInput")
    wv_d = nc.dram_tensor("wv", [L, P, CCH * C], dt.bfloat16, kind="ExternalInput")
    wo_d = nc.dram_tensor("wo", [L, P, CCH * C], dt.bfloat16, kind="ExternalInput")
    wg_d = nc.dram_tensor("wg", [P, CCH * C], dt.bfloat16, kind="ExternalInput")
    wf_d = nc.dram_tensor("wf", [P, 2 * C], dt.bfloat16, kind="ExternalInput")
    y_d = nc.dram_tensor("y", [ROWS, C], dt.float32, kind="ExternalOutput")
    if use_gamma:
        gam_d = nc.dram_tensor("gam", [L + 1, P, C], dt.bfloat16, kind="ExternalInput")
    if use_beta:
        bet_d = nc.dram_tensor("bet", [L + 1, P, C], dt.bfloat16, kind="ExternalInput")
    if use_bo:
        bo_d = nc.dram_tensor("bob", [L, P, C], dt.float32, kind="ExternalInput")
    if use_bg:
        bg_d = nc.dram_tensor("bgc", [P, CCH], dt.float32, kind="ExternalInput")
    if use_bf:
        bf_d = nc.dram_tensor("bfb", [P, C], dt.float32, kind="ExternalInput")

    dbg = {}
    if debug:
        for nm, w in [("hsb", RT * C), ("htsb", CCH * ROWS), ("qtsb", CCH * ROWS),
                      ("kstg", CCH * ROWS), ("ktsb", CCH * S), ("osc", CCH * ROWS),
                      ("xsb2", RT * C)]:
            dtt = dt.float32 if nm == "xsb2" else dt.bfloat16
            dbg[nm] = nc.dram_tensor("dbg_" + nm, [P, w], dtt, kind="ExternalOutput")
        dbg["vsb"] = nc.dram_tensor("dbg_vsb", [P, KT * VW], dt.bfloat16,
                                    kind="ExternalOutput")
        dbg["esb0"] = nc.dram_tensor("dbg_esb0", [P, KT * ROWS], dt.bfloat16,
                                     kind="ExternalOutput")
        dbg["av0"] = nc.dram_tensor("dbg_av0", [CH + 1, ROWS], dt.float32,
                                    kind="ExternalOutput")

    # collective bounce buffers (DRAM, internal), separate per K / V so the
    # K AllGather completes (and unblocks score matmuls) while V's runs
    kin_k = [nc.dram_tensor(f"kin_k{i}", [P, CCH * ROWS], dt.bfloat16, kind="Internal")
             for i in range(2)]
    kout_k = [nc.dram_tensor(f"kout_k{i}", [GROUP, P, CCH * ROWS], dt.bfloat16,
                             kind="Internal") for i in range(2)]
    kin_v = [nc.dram_tensor(f"kin_v{i}", [P, KTO * VW], dt.bfloat16, kind="Internal")
             for i in range(2)]
    kout_v = [nc.dram_tensor(f"kout_v{i}", [GROUP, P, KTO * VW], dt.bfloat16,
                             kind="Internal") for i in range(2)]

    # ---- persistent SBUF ----
    XSB = nc.alloc_sbuf_tensor("xsb", [P, RT * C], dt.float32).ap()
    HSB = nc.alloc_sbuf_tensor("hsb", [P, RT * C], dt.bfloat16).ap()
    HTSB = nc.alloc_sbuf_tensor("htsb", [P, CCH * ROWS], dt.bfloat16).ap()
    QTSB = nc.alloc_sbuf_tensor("qtsb", [P, CCH * ROWS], dt.bfloat16).ap()
    KSTG = nc.alloc_sbuf_tensor("kstg", [P, CCH * ROWS], dt.bfloat16).ap()
    KTSB = nc.alloc_sbuf_tensor("ktsb", [P, CCH * S], dt.bfloat16).ap()
    VSTG = nc.alloc_sbuf_tensor("vstg", [P, KTO * VW], dt.bfloat16).ap()
    VSB = nc.alloc_sbuf_tensor("vsb", [P, KT * VW], dt.bfloat16).ap()
    OSC = nc.alloc_sbuf_tensor("osc", [P, CCH * ROWS], dt.bfloat16).ap()
    FFSB = nc.alloc_sbuf_tensor("ffsb", [P, 2 * ROWS], dt.bfloat16).ap()
    ONES = nc.alloc_sbuf_tensor("ones", [P, CH], dt.bfloat16).ap()
    RECF = nc.alloc_sbuf_tensor("recf", [P, 2 * ROWS], dt.float32).ap()
    RECB = nc.alloc_sbuf_tensor("recb", [P, 2 * ROWS], dt.bfloat16).ap()

    with tile.TileContext(nc) as tc:
        with (
            tc.tile_pool(name="wpool", bufs=2) as wpool,
            tc.tile_pool(name="epool", bufs=2) as epool,
            tc.tile_pool(name="small", bufs=2) as small,
            tc.tile_pool(name="gpool", bufs=2) as gpool,
            tc.tile_pool(name="mmps", bufs=2, space="PSUM") as mmps,
            tc.tile_pool(name="scps", bufs=3, space="PSUM") as scps,
        ):
            # one-time init
            nc.vector.memset(ONES, 1.0)
            vview = VSTG.rearrange("p (kt h c) -> p kt h c", kt=KTO, h=H)
            nc.vector.memset(vview[:, :, :, CH:CH + 1], 1.0)
            nc.sync.dma_start(XSB.rearrange("p (rt c) -> p rt c", rt=RT),
                              x_d.ap().rearrange("(rt p) c -> p rt c", p=P))

            def layer_norm(li):
                """x (XSB) -> h bf16 (HSB), then h^T (HTSB)."""
                MV = small.tile([P, 2 * RT], dt.float32, tag="mv")
                for rt in range(RT):
                    st6 = small.tile([P, 6], dt.float32, tag="st6")
                    nc.vector.bn_stats(st6[:], XSB[:, rt * C:(rt + 1) * C])
                    nc.vector.bn_aggr(MV[:, 2 * rt:2 * rt + 2], st6[:])
                # rstd = rsqrt(var + EPS) on DVE (quake + 2 Newton steps)
                var = MV[:].rearrange("p (rt two) -> p two rt", two=2)[:, 1, :]
                VT = small.tile([P, RT], dt.float32, tag="vt")
                VH = small.tile([P, RT], dt.float32, tag="vh")
                KC = small.tile([P, RT], dt.int32, tag="kc")
                R0 = small.tile([P, RT], dt.int32, tag="r0")
                nc.vector.tensor_scalar(VT[:], var, EPS, None, OP.add)
                nc.vector.tensor_scalar(VH[:], VT[:], 0.5, None, OP.mult)
                nc.vector.memset(KC[:], SQRT_K)
                nc.vector.tensor_scalar(R0[:], VT[:].bitcast(dt.int32), 1, None,
                                        OP.logical_shift_right)
                nc.vector.scalar_tensor_tensor(R0[:], KC[:], 0, R0[:],
                                               OP.bypass, OP.subtract)
                r = R0[:].bitcast(dt.float32)
                for _ in range(2):
                    A = small.tile([P, RT], dt.float32, tag="nra")
                    Cc = small.tile([P, RT], dt.float32, tag="nrc")
                    Rn = small.tile([P, RT], dt.float32, tag="nrr")
                    nc.vector.tensor_mul(A[:], r, r)
                    nc.vector.tensor_mul(A[:], A[:], VH[:])
                    nc.vector.tensor_scalar(Cc[:], A[:], -1.0, 1.5, OP.mult, OP.add)
                    nc.vector.tensor_mul(Rn[:], r, Cc[:])
                    r = Rn[:]
                if use_gamma:
                    GT = gpool.tile([P, C], dt.bfloat16, tag="gam")
                    nc.sync.dma_start(GT[:], gam_d.ap()[li])
                if use_beta:
                    BT = gpool.tile([P, C], dt.bfloat16, tag="bet")
                    nc.sync.dma_start(BT[:], bet_d.ap()[li])
                for rt in range(RT):
                    dst = HSB[:, rt * C:(rt + 1) * C]
                    nc.vector.tensor_scalar(dst, XSB[:, rt * C:(rt + 1) * C],
                                            MV[:, 2 * rt:2 * rt + 1],
                                            r[:, rt:rt + 1],
                                            OP.subtract, OP.mult)
                    if use_gamma:
                        nc.vector.tensor_mul(dst, dst, GT[:])
                    if use_beta:
                        nc.vector.tensor_add(dst, dst, BT[:])
                # h^T via x-bar transpose
                htv = HTSB.rearrange("p (cc r) -> p cc r", cc=CCH)
                for rt in range(RT):
                    nc.sync.dma_start_transpose(
                        htv[:, :, rt * P:(rt + 1) * P],
                        HSB[:, rt * C:(rt + 1) * C])

            def proj_t(wt, dst_col):
                """dst[:, mc*ROWS...] = W^T-style projection -> transposed
                output [c, rows] written via dst_col(mc) APs."""
                for mc in range(CCH):
                    ps = mmps.tile([P, ROWS], dt.float32, tag="mm")
                    for kc in range(CCH):
                        nc.tensor.matmul(
                            ps[:],
                            lhsT=wt[:, kc * C + mc * P: kc * C + (mc + 1) * P],
                            rhs=HTSB[:, kc * ROWS:(kc + 1) * ROWS],
                            start=(kc == 0), stop=(kc == CCH - 1))
                    nc.vector.tensor_copy(dst_col(mc), ps[:])

            def attn_layer(li):
                WQ = wpool.tile([P, CCH * C], dt.bfloat16, tag="wq")
                WK = wpool.tile([P, CCH * C], dt.bfloat16, tag="wk")
                WV = wpool.tile([P, CCH * C], dt.bfloat16, tag="wv")
                WO = wpool.tile([P, CCH * C], dt.bfloat16, tag="wo")
                nc.sync.dma_start(WK[:], wk_d.ap()[li])
                nc.sync.dma_start(WV[:], wv_d.ap()[li])
                nc.sync.dma_start(WQ[:], wq_d.ap()[li])
                nc.sync.dma_start(WO[:], wo_d.ap()[li])

                layer_norm(li)
                if debug and li == 0:
                    nc.sync.dma_start(dbg["hsb"].ap(), HSB)
                    nc.sync.dma_start(dbg["htsb"].ap(), HTSB)

                kin_ki, kout_ki = kin_k[li % 2], kout_k[li % 2]
                kin_vi, kout_vi = kin_v[li % 2], kout_v[li % 2]

                # K^T (own shard), then its own AllGather so score matmuls can
                # start while the V AllGather is still in flight
                proj_t(WK[:], lambda mc: KSTG[:, mc * ROWS:(mc + 1) * ROWS])
                nc.sync.dma_start(kin_ki.ap(), KSTG)
                if fake_ag:
                    for r in range(GROUP):
                        nc.sync.dma_start(kout_ki.ap()[r], kin_ki.ap())
                else:
                    nc.gpsimd.collective_compute(
                        "AllGather", mybir.AluOpType.bypass, replica_groups=groups,
                        ins=[kin_ki.ap().opt()], outs=[kout_ki.ap().opt()])

                # V (own shard, natural layout + ones cols)
                vdst = VSTG.rearrange("p (kt h c) -> p kt h c", kt=KTO, h=H)
                for kt in range(KTO):
                    ps = mmps.tile([P, C], dt.float32, tag="mm")
                    for kc in range(CCH):
                        nc.tensor.matmul(
                            ps[:],
                            lhsT=HTSB[:, kc * ROWS + kt * P: kc * ROWS + (kt + 1) * P],
                            rhs=WV[:, kc * C:(kc + 1) * C],
                            start=(kc == 0), stop=(kc == CCH - 1))
                    nc.vector.tensor_copy(
                        vdst[:, kt, :, 0:CH],
                        ps[:].rearrange("p (h c) -> p h c", h=H))
                nc.sync.dma_start(kin_vi.ap(), VSTG)
                if fake_ag:
                    for r in range(GROUP):
                        nc.sync.dma_start(kout_vi.ap()[r], kin_vi.ap())
                else:
                    nc.gpsimd.collective_compute(
                        "AllGather", mybir.AluOpType.bypass, replica_groups=groups,
                        ins=[kin_vi.ap().opt()], outs=[kout_vi.ap().opt()])

                # Q^T (overlaps with the collectives)
                proj_t(WQ[:], lambda mc: QTSB[:, mc * ROWS:(mc + 1) * ROWS])

                # unload gathered K^T (first; scores need it) then V
                ktv = KTSB.rearrange("p (cc k) -> p cc k", cc=CCH)
                for r in range(GROUP):
                    nc.sync.dma_start(ktv[:, :, r * ROWS:(r + 1) * ROWS],
                                      kout_ki.ap()[r])
                for r in range(GROUP):
                    nc.sync.dma_start(VSB[:, r * KTO * VW:(r + 1) * KTO * VW],
                                      kout_vi.ap()[r])

                if debug and li == 0:
                    nc.sync.dma_start(dbg["qtsb"].ap(), QTSB)
                    nc.sync.dma_start(dbg["kstg"].ap(), KSTG)
                    nc.sync.dma_start(dbg["ktsb"].ap(), KTSB)
                    nc.sync.dma_start(dbg["vsb"].ap(), VSB)

                # attention, two heads at a time (row groups 0-63 / 64-127
                # overlap in the PE array via tile_position)
                def normalize(h, av):
                    sub = (h % 2) * CH
                    cc_h = h // 2
                    slot = h % 2
                    rf = RECF[CH:CH + 1, slot * ROWS:(slot + 1) * ROWS]
                    rb = RECB[CH:CH + 1, slot * ROWS:(slot + 1) * ROWS]
                    nc.vector.reciprocal(rf, av[CH:CH + 1, :])
                    nc.vector.tensor_copy(rb, rf)
                    bc = scps.tile([P, ROWS], dt.float32, tag="sc")
                    nc.tensor.matmul(bc[0:CH, :], lhsT=ONES[CH:CH + 1, :],
                                     rhs=rb, start=True, stop=True)
                    bcs = small.tile([CH, ROWS], dt.bfloat16, tag="bcs")
                    nc.vector.tensor_copy(bcs[:], bc[0:CH, :])
                    nc.vector.tensor_mul(
                        OSC[sub:sub + CH, cc_h * ROWS:(cc_h + 1) * ROWS],
                        av[0:CH, :], bcs[:])

                def emit_av(h, esb):
                    """AV accumulation (incl. softmax sums via the ones
                    column) + normalization for head h."""
                    av = mmps.tile([CH + 1, ROWS], dt.float32, tag="mm")
                    for kt in range(KT):
                        nc.tensor.matmul(
                            av[:],
                            lhsT=VSB[:, kt * VW + h * (CH + 1): kt * VW + (h + 1) * (CH + 1)],
                            rhs=esb[:, kt * ROWS:(kt + 1) * ROWS],
                            start=(kt == 0), stop=(kt == KT - 1))
                    normalize(h, av)

                # software pipeline: scores/exp of head h overlap the AV
                # matmul chain of head h-1 (otherwise ACT idles behind the
                # in-order PE queue for every AV chain)
                pending = None
                for h in range(H):
                    sub = (h % 2) * CH
                    cc_h = h // 2
                    esb = epool.tile([P, KT * ROWS], dt.bfloat16, tag="esb")
                    for pair in range(KT // 2):
                        sp = scps.tile([P, 2 * ROWS], dt.float32, tag="sc")
                        for j in range(2):
                            kt = 2 * pair + j
                            nc.tensor.matmul(
                                sp[:, j * ROWS:(j + 1) * ROWS],
                                lhsT=KTSB[sub:sub + CH,
                                          cc_h * S + kt * P: cc_h * S + (kt + 1) * P],
                                rhs=QTSB[sub:sub + CH,
                                         cc_h * ROWS:(cc_h + 1) * ROWS],
                                start=True, stop=True)
                        nc.scalar.activation(
                            esb[:, pair * 2 * ROWS:(pair + 1) * 2 * ROWS],
                            sp[:], mybir.ActivationFunctionType.Exp,
                            scale=1.0 / np.sqrt(CH))
                    if pending is not None:
                        emit_av(*pending)
                    pending = (h, esb)
                emit_av(*pending)

                # out-projection + residual (natural layout)
                if use_bo:
                    BO = gpool.tile([P, C], dt.float32, tag="bo")
                    nc.sync.dma_start(BO[:], bo_d.ap()[li])
                for rt in range(RT):
                    ps = mmps.tile([P, C], dt.float32, tag="mm")
                    for kc in range(CCH):
                        nc.tensor.matmul(
                            ps[:],
                            lhsT=OSC[:, kc * ROWS + rt * P: kc * ROWS + (rt + 1) * P],
                            rhs=WO[:, kc * C:(kc + 1) * C],
                            start=(kc == 0), stop=(kc == CCH - 1))
                    dst = XSB[:, rt * C:(rt + 1) * C]
                    nc.vector.tensor_add(dst, ps[:], dst)
                    if use_bo:
                        nc.vector.tensor_add(dst, dst, BO[:])
                if debug and li == 0:
                    nc.sync.dma_start(dbg["osc"].ap(), OSC)
                    nc.sync.dma_start(dbg["xsb2"].ap(), XSB)

            for li in range(n_layers):
                attn_layer(li)

            # ---- FFN ----
            layer_norm(L)
            WG = wpool.tile([P, CCH * C], dt.bfloat16, tag="wq")
            WF = wpool.tile([P, 2 * C], dt.bfloat16, tag="wf")
            nc.sync.dma_start(WG[:], wg_d.ap())
            nc.sync.dma_start(WF[:], wf_d.ap())
            if use_bg:
                BG = gpool.tile([P, CCH], dt.float32, tag="bg")
                nc.sync.dma_start(BG[:], bg_d.ap())
            AGT = small.tile([P, 2 * ROWS], dt.bfloat16, tag="ffa")
            GGT = small.tile([P, 2 * ROWS], dt.bfloat16, tag="ffg")
            for mg in range(CCH):
                ps = mmps.tile([P, ROWS], dt.float32, tag="mm")
                for kc in range(CCH):
                    nc.tensor.matmul(
                        ps[:],
                        lhsT=WG[:, kc * C + mg * P: kc * C + (mg + 1) * P],
                        rhs=HTSB[:, kc * ROWS:(kc + 1) * ROWS],
                        start=(kc == 0), stop=(kc == CCH - 1))
                dst = (AGT if mg < 2 else GGT)[:, (mg % 2) * ROWS:(mg % 2 + 1) * ROWS]
                if use_bg:
                    nc.vector.tensor_scalar(dst, ps[:], BG[:, mg:mg + 1], None,
                                            mybir.AluOpType.add)
                else:
                    nc.vector.tensor_copy(dst, ps[:])
            K1 = 0.7978845608
            for j in range(2):
                ga = GGT[:, j * ROWS:(j + 1) * ROWS]
                aa = AGT[:, j * ROWS:(j + 1) * ROWS]
                SQ = small.tile([P, ROWS], dt.float32, tag="sq")
                WT = small.tile([P, ROWS], dt.float32, tag="wt")
                VV = small.tile([P, ROWS], dt.float32, tag="vv")
                TT = small.tile([P, ROWS], dt.float32, tag="tt")
                HT2 = small.tile([P, ROWS], dt.bfloat16, tag="ht2")
                PP = small.tile([P, ROWS], dt.bfloat16, tag="pp")
                nc.scalar.activation(SQ[:], ga, mybir.ActivationFunctionType.Square)
                nc.vector.tensor_scalar(WT[:], SQ[:], K1 * 0.044715, K1,
                                        mybir.AluOpType.mult, mybir.AluOpType.add)
                nc.vector.tensor_mul(VV[:], ga, WT[:])
                nc.scalar.activation(TT[:], VV[:], mybir.ActivationFunctionType.Tanh)
                nc.vector.tensor_scalar(HT2[:], TT[:], 0.5, 0.5,
                                        mybir.AluOpType.mult, mybir.AluOpType.add)
                nc.vector.tensor_mul(PP[:], aa, ga)
                nc.vector.tensor_mul(FFSB[:, j * ROWS:(j + 1) * ROWS], PP[:], HT2[:])
            if use_bf:
                BF = gpool.tile([P, C], dt.float32, tag="bf")
                nc.sync.dma_start(BF[:], bf_d.ap())
            for rt in range(RT):
                ps = mmps.tile([P, C], dt.float32, tag="mm")
                for kc in range(2):
                    nc.tensor.matmul(
                        ps[:],
                        lhsT=FFSB[:, kc * ROWS + rt * P: kc * ROWS + (rt + 1) * P],
                        rhs=WF[:, kc * C:(kc + 1) * C],
                        start=(kc == 0), stop=(kc == 1))
                OUT = small.tile([P, C], dt.float32, tag="out")
                nc.vector.tensor_add(OUT[:], ps[:], XSB[:, rt * C:(rt + 1) * C])
                if use_bf:
                    nc.vector.tensor_add(OUT[:], OUT[:], BF[:])
                nc.sync.dma_start(y_d.ap()[rt * P:(rt + 1) * P, :], OUT[:])

    nc.compile()
    return nc


def kernel(x, ln_gamma, ln_beta, Wq, Wk, Wv, Wo, bo, Wg, bg, Wf, bf):
    x = np.asarray(x, np.float32)
    ln_gamma = np.asarray(ln_gamma, np.float32)
    ln_beta = np.asarray(ln_beta, np.float32)
    Wq, Wk, Wv, Wo = (np.asarray(w, np.float32) for w in (Wq, Wk, Wv, Wo))
    bo = np.asarray(bo, np.float32)
    Wg, Wf = np.asarray(Wg, np.float32), np.asarray(Wf, np.float32)
    bg, bf = np.asarray(bg, np.float32), np.asarray(bf, np.float32)

    use_gamma = not np.all(ln_gamma == 1.0)
    use_beta = not np.all(ln_beta == 0.0)
    use_bo = not np.all(bo == 0.0)
    use_bg = not np.all(bg == 0.0)
    use_bf = not np.all(bf == 0.0)
    flags = (use_gamma, use_beta, use_bo, use_bg, use_bf)

    if flags not in _CACHE:
        _CACHE[flags] = _build(flags)
    nc = _CACHE[flags]

    wq_h = np.stack([_swz(Wq[l], CCH) for l in range(L)]).astype(BF16)
    wk_h = np.stack([_swz(Wk[l], CCH) for l in range(L)]).astype(BF16)
    wv_h = np.stack([_swz(Wv[l], CCH) for l in range(L)]).astype(BF16)
    wo_h = np.stack([_swz(Wo[l], CCH) for l in range(L)]).astype(BF16)
    wg_h = _swz(Wg, CCH).astype(BF16)
    wf_h = _swz(Wf, 2).astype(BF16)

    xf = x.reshape(B * S, C)
    base = {
        "wq": wq_h, "wk": wk_h, "wv": wv_h, "wo": wo_h,
        "wg": wg_h, "wf": wf_h,
    }
    if use_gamma:
        base["gam"] = np.ascontiguousarray(
            np.broadcast_to(ln_gamma[:, None, :], (L + 1, P, C))).astype(BF16)
    if use_beta:
        base["bet"] = np.ascontiguousarray(
            np.broadcast_to(ln_beta[:, None, :], (L + 1, P, C))).astype(BF16)
    if use_bo:
        base["bob"] = np.ascontiguousarray(
            np.broadcast_to(bo[:, None, :], (L, P, C))).astype(np.float32)
    if use_bg:
        base["bgc"] = np.ascontiguousarray(bg.reshape(CCH, P).T).astype(np.float32)
    if use_bf:
        base["bfb"] = np.ascontiguousarray(
            np.broadcast_to(bf[None, :], (P, C))).astype(np.float32)

    in_maps = []
    for c in range(N_CORES):
        m = dict(base)
        m["x"] = np.ascontiguousarray(xf[c * ROWS:(c + 1) * ROWS])
        in_maps.append(m)

    from concourse.bass_utils import run_bass_kernel_spmd
    res = run_bass_kernel_spmd(nc, in_maps, core_ids=list(range(N_CORES)))
    out = np.concatenate([res.results[c]["y"] for c in range(N_CORES)], axis=0)
    return out.reshape(B, S, C).astype(np.float32)

